# revision 1
# baseline (speedup 1.0000x reference)
"""N-gram embedding lookup kernel for Trainium2 (8 NeuronCores, Bass/Tile).

Problem: for each token x[b,s] (vocab 50000), gather precomputed n-gram
hash ids for orders 1..3 (12+11+10 slots), gather embedding rows from
three tables (1001/10001/50001 x 256 fp32), masked-mean each order,
concat to 768 dims; tokens x<4 take tab0[x] instead.

Environment constraints (verified on HW this session):
 - no HIPI ucode => custom bulk-gather (InstDMAGatherAnt) is unavailable
   (NRT_EXEC_UNIT_UNRECOVERABLE when executed);
 - walrus-native indirect DMA (InstDMACopy + dynamic AP) gathers exactly
   ONE row per partition per instruction (extra offset-AP indices are
   ignored; the descriptor reads out-row-size contiguous bytes from the
   single per-partition index), and each instruction serializes ~1.7us
   (bypass) / ~2.2us (CCE-add) of Pool-engine SWDGE descriptor
   generation. That serial gen time dominates; num_swdge_queues,
   acc dtype, and chain splitting measurably do NOT change it, so the
   only lever is issuing FEWER indirect-DMA instructions.

Design (measured ~325us/core vs ~970us for the naive 476-instruction
data-parallel layout; rel err 1.66e-3):
 - host: dedup x to unique words, sort by word length (cnt1 desc), deal
   round-robin to the 8 cores => every 128-token group holds words of
   nearly equal length, and per-group slot counts (the "profile")
   shrink from (12,11,10) to the group's actual max cnt per order. The
   Bass program is compiled per profile (cached) and outputs are
   scattered back to token positions on host.
 - host: per-token meta rows (ids/cnts, int32 [P, G*40]) are shipped
   directly; specials (x<4) are folded into the tables as 4 appended
   rows so no separate patch pass exists.
 - chip, order 1 (1001-row table): offloaded off the Pool engine
   entirely - DVE builds per-token bucket histograms via f16 iota
   is_equal one-hots, PE transposes them and matmuls against the
   SBUF-resident padded table, accumulating in PSUM.
 - chip, orders 2/3: profile[g][o] independent bypass indirect-DMA
   gathers (bf16 rows -> f32 cast in the DMA) into slot slabs, then one
   DVE tensor_reduce + 1/cnt scale per (group, order). Slabs rotate
   5-deep; each group's reduce is issued two groups late so the Pool
   engine never waits on DVE (measured -18us vs one group late).
 - per-group stores overlap the Pool gather stream on the SP engine.
"""
import numpy as np
import ml_dtypes
from contextlib import ExitStack

from concourse import bacc, bass, mybir, tile
from concourse.bass_utils import run_bass_kernel_spmd

BF16 = ml_dtypes.bfloat16

NCORES = 8
B, S = 8, 2048
TOK = B * S
TPC = TOK // NCORES          # 2048 tokens per core
P = 128
G = TPC // P                 # 16 groups
EMB = 256
V = 50000
LS = (12, 11, 10)
COLBASE = (0, 12, 23)        # meta col of slot 0 per order
CNTCOL = 33                  # meta cols 33..35 = cnt1..3
MW = 40                      # meta row width (int32)
NQ = 1                       # SWDGE queues to spray across
OFFLOAD1 = True              # compute order-1 via DVE one-hots + PE matmul
META_HOST = True             # ship per-token meta rows from host (no gather)
SLOTS = True                 # bypass gathers into slot slabs + DVE reduce
W1 = 1024                    # order-1 bucket space (1005 used, padded)
NB1 = W1 // 128


def _build(profile, num_swdge_queues=NQ, unroll=1, offload1=OFFLOAD1,
           meta_host=META_HOST, cce_off=False, acc_bf16=False, rot=0,
           slots=SLOTS):
    G = len(profile)
    TPC = G * P
    i32, f32, bf16 = mybir.dt.int32, mybir.dt.float32, mybir.dt.bfloat16
    f16 = mybir.dt.float16
    nc = bacc.Bacc("TRN2", target_bir_lowering=False, debug=False,
                   num_devices=NCORES, num_swdge_queues=num_swdge_queues)

    if meta_host:
        d_metaT = nc.dram_tensor("metaT", [P, G * MW], i32,
                                 kind="ExternalInput")
    else:
        d_meta = nc.dram_tensor("metaI", [V, MW], i32, kind="ExternalInput")
        d_xpg = nc.dram_tensor("xpg", [P, G], i32, kind="ExternalInput")
    d_tabs = [nc.dram_tensor(f"tab{o+1}z", [(1005, 10005, 50005)[o], EMB],
                             bf16, kind="ExternalInput") for o in range(3)]
    d_out = nc.dram_tensor("out", [TPC, 768], f32, kind="ExternalOutput")
    if offload1:
        d_iota = nc.dram_tensor("iotaH", [P, W1], f16, kind="ExternalInput")
        d_tab1p = nc.dram_tensor("tab1p", [W1, EMB], bf16,
                                 kind="ExternalInput")
        d_id = nc.dram_tensor("eyeH", [P, P], f16, kind="ExternalInput")

    swdge_q = [0]

    def spray(inst):
        # round-robin independent DMAs across SWDGE queues; keep each
        # accumulation chain on one queue (callers rotate per chain)
        if num_swdge_queues > 1:
            q = swdge_q[0] % num_swdge_queues
            if q:
                inst.ins.queue = f"qPoolDynamic{q}"
        return inst

    def next_q():
        swdge_q[0] += 1

    with ExitStack() as ctx:
        tc = ctx.enter_context(tile.TileContext(nc))
        pool = ctx.enter_context(tc.tile_pool(name="sbuf", bufs=1))

        t_xpg = pool.tile([P, G], i32)
        t_meta = pool.tile([P, G * MW], i32)
        o_lo = 1 if offload1 else 0
        acc_dt = bf16 if acc_bf16 else f32
        rot_tiles = [pool.tile([P, EMB], acc_dt, name=f"rot_{i}")
                     for i in range(rot)] if rot else None
        if slots:
            lmax = {o: max(p[o] for p in profile) for o in range(o_lo, 3)}
            slabs = {o: [pool.tile([P, lmax[o] * EMB], f32,
                                   name=f"slab_{o}_{i}") for i in range(5)]
                     for o in range(o_lo, 3)}
            accs = None
        else:
            accs = {(g, o): (rot_tiles[(g * 3 + o) % rot] if rot else
                             pool.tile([P, EMB], acc_dt, name=f"acc_{g}_{o}"))
                    for g in range(G) for o in range(o_lo, 3)}
        t_cntf = pool.tile([P, G * 3], f32)
        t_rcp = pool.tile([P, G * 3], f32)
        t_out = pool.tile([P, G * 768], f32)
        if offload1:
            psum = ctx.enter_context(
                tc.tile_pool(name="psum", bufs=1, space="PSUM"))
            t_iota = pool.tile([P, W1], f16)
            t_tab1 = pool.tile([P, NB1 * EMB], bf16)
            t_id = pool.tile([P, P], f16)
            t_ids1f = pool.tile([P, G * 12], f16)
            t_H = [pool.tile([P, W1], f16, name=f"H{i}") for i in range(2)]
            t_oh = [pool.tile([P, W1], f16, name=f"oh{i}") for i in range(2)]
            t_HT = [pool.tile([P, NB1 * P], bf16, name=f"HT{i}")
                    for i in range(2)]
            ps_T = [psum.tile([P, P], f16, name=f"psT{i}") for i in range(2)]
            ps_E = [psum.tile([P, EMB], f32, name=f"psE{i}")
                    for i in range(4)]
            nc.sync.dma_start(out=t_iota[:], in_=d_iota[:])
            nc.sync.dma_start(
                out=bass.AP(t_tab1[:].tensor, 0,
                            [t_tab1[:].ap[0], [EMB, NB1], [1, EMB]]),
                in_=bass.AP(d_tab1p, 0, [[EMB, P], [P * EMB, NB1], [1, EMB]]))
            nc.sync.dma_start(out=t_id[:], in_=d_id[:])

        if unroll > 1:
            # hardware loop for benchmarking: body is idempotent
            ctx.enter_context(tc.For_i(0, unroll))

        if True:
            if meta_host:
                nc.sync.dma_start(out=t_meta[:], in_=d_metaT[:])
            else:
                nc.sync.dma_start(out=t_xpg[:], in_=d_xpg[:])
                # ---- meta gathers: one per group, row [40 int32] per token
                for g in range(G):
                    spray(nc.gpsimd.indirect_dma_start(
                        out=t_meta[:, g * MW:(g + 1) * MW],
                        out_offset=None,
                        in_=d_meta[:],
                        in_offset=bass.IndirectOffsetOnAxis(
                            ap=t_xpg[:, g:g + 1], axis=0)))
                    next_q()

            # ---- 1/cnt
            nc.vector.tensor_copy(
                out=t_cntf[:],
                in_=bass.AP(t_meta[:].tensor, CNTCOL,
                            [t_meta[:].ap[0], [MW, G], [1, 3]]))
            nc.vector.reciprocal(out=t_rcp[:], in_=t_cntf[:])

            # ---- per-group pipeline: Pool gathers / DVE+PE order-1 /
            #      DVE reduces+scales / SP store.  Issue order per group
            #      keeps slab/H/psum rotation windows correct and lets
            #      every engine run concurrently.
            rix = [0]

            def emit_tail(g):
                # DVE: reduce slots (v4) / scale into output tile
                for o in range(o_lo, 3):
                    out_ap = t_out[:, g * 768 + o * 256:
                                   g * 768 + (o + 1) * 256]
                    in1 = bass.AP(t_rcp[:].tensor, g * 3 + o,
                                  [t_rcp[:].ap[0], [0, 256]])
                    if slots:
                        L = profile[g][o]
                        slab = slabs[o][g % 5]
                        if L == 1:
                            nc.vector.tensor_tensor(
                                out=out_ap, in0=slab[:, 0:EMB], in1=in1,
                                op=mybir.AluOpType.mult)
                        else:
                            nc.vector.tensor_reduce(
                                out=out_ap,
                                in_=bass.AP(slab[:].tensor, 0,
                                            [slab[:].ap[0], [1, EMB],
                                             [EMB, L]]),
                                axis=mybir.AxisListType.X,
                                op=mybir.AluOpType.add, opt_input=False)
                            nc.vector.tensor_tensor(out=out_ap, in0=out_ap,
                                                    in1=in1,
                                                    op=mybir.AluOpType.mult)
                    else:
                        acc = accs[(g, o)]
                        nc.vector.tensor_tensor(out=out_ap, in0=acc[:],
                                                in1=in1,
                                                op=mybir.AluOpType.mult)
                # store: SBUF (p, g, 768) -> DRAM row g*128+p
                nc.sync.dma_start(
                    out=bass.AP(d_out, g * P * 768, [[768, P], [1, 768]]),
                    in_=t_out[:, g * 768:(g + 1) * 768])

            for g in range(G):
                # Pool: embedding gathers for orders 2..3
                for o in range(o_lo, 3):
                    L = profile[g][o]
                    for s in range(L):
                        col = g * MW + COLBASE[o] + s
                        if slots:
                            slab = slabs[o][g % 5]
                            dst = slab[:, s * EMB:(s + 1) * EMB]
                            op = mybir.AluOpType.bypass
                        elif rot:
                            dst = rot_tiles[rix[0] % rot][:]
                            rix[0] += 1
                            op = mybir.AluOpType.bypass
                        else:
                            dst = accs[(g, o)][:]
                            op = (mybir.AluOpType.bypass
                                  if (s == 0 or cce_off)
                                  else mybir.AluOpType.add)
                        spray(nc.gpsimd.indirect_dma_start(
                            out=dst,
                            out_offset=None,
                            in_=d_tabs[o][:],
                            in_offset=bass.IndirectOffsetOnAxis(
                                ap=t_meta[:, col:col + 1], axis=0),
                            compute_op=op))
                    next_q()

                if g > 1:
                    emit_tail(g - 2)
                if g == G - 1 and g >= 1:
                    emit_tail(g - 1)

                if offload1:
                    # order-1 via one-hot histogram + PE matmul
                    s1 = profile[g][0]
                    H = t_H[g % 2]
                    # ids (int32 meta cols) -> f16, exact for ids <= 2048
                    nc.vector.tensor_copy(
                        out=t_ids1f[:, g * 12:g * 12 + s1],
                        in_=t_meta[:, g * MW:g * MW + s1])
                    for s in range(s1):
                        idb = bass.AP(t_ids1f[:].tensor, g * 12 + s,
                                      [t_ids1f[:].ap[0], [0, W1]])
                        if s == 0:
                            nc.vector.tensor_tensor(
                                out=H[:], in0=t_iota[:], in1=idb,
                                op=mybir.AluOpType.is_equal)
                        else:
                            oh = t_oh[g % 2]
                            nc.vector.tensor_tensor(
                                out=oh[:], in0=t_iota[:], in1=idb,
                                op=mybir.AluOpType.is_equal)
                            nc.vector.tensor_tensor(
                                out=H[:], in0=H[:], in1=oh[:],
                                op=mybir.AluOpType.add)
                    HT = t_HT[g % 2]
                    for k in range(NB1):
                        pT = ps_T[k % 2]
                        nc.tensor.transpose(
                            pT[:], H[:, k * P:(k + 1) * P], t_id[:])
                        nc.vector.tensor_copy(
                            out=HT[:, k * P:(k + 1) * P], in_=pT[:])
                    pE = ps_E[g % 4]
                    for k in range(NB1):
                        nc.tensor.matmul(
                            pE[:],
                            lhsT=HT[:, k * P:(k + 1) * P],
                            rhs=t_tab1[:, k * EMB:(k + 1) * EMB],
                            start=(k == 0), stop=(k == NB1 - 1))
                    in1 = bass.AP(t_rcp[:].tensor, g * 3 + 0,
                                  [t_rcp[:].ap[0], [0, EMB]])
                    nc.vector.tensor_tensor(
                        out=t_out[:, g * 768:g * 768 + EMB],
                        in0=pE[:], in1=in1, op=mybir.AluOpType.mult)

            emit_tail(G - 1)

    return nc


_NC_CACHE = {}


def _get_nc(profile, nq=NQ, unroll=1, offload1=OFFLOAD1,
            meta_host=META_HOST, **kw):
    key = (profile, nq, unroll, offload1, meta_host, tuple(sorted(kw.items())))
    if key not in _NC_CACHE:
        nc = _build(profile, num_swdge_queues=nq, unroll=unroll,
                    offload1=offload1, meta_host=meta_host, **kw)
        nc.finalize()
        _NC_CACHE[key] = nc
    return _NC_CACHE[key]


def _prep(inputs):
    tab0 = np.asarray(inputs['tab0'], np.float32)
    tabs = [np.asarray(inputs[f'tab{o+1}'], np.float32) for o in range(3)]
    ids = [np.asarray(inputs[f'ids{o+1}'], np.int64) for o in range(3)]
    cnt = [np.asarray(inputs[f'cnt{o+1}'], np.int64) for o in range(3)]

    meta = np.zeros((V, MW), np.int32)
    for o in range(3):
        meta[:, COLBASE[o]:COLBASE[o] + LS[o]] = ids[o]
        meta[:, CNTCOL + o] = cnt[o]
    # specials: slot 0 -> appended per-special row, others 0, cnt 1
    meta[:4, :CNTCOL] = 0
    meta[:4, CNTCOL:CNTCOL + 3] = 1
    nrows = (1001, 10001, 50001)
    for o in range(3):
        meta[:4, COLBASE[o]] = nrows[o] + np.arange(4)

    shared = {'metaI': meta}
    for o in range(3):
        tz = np.zeros((nrows[o] + 4, EMB), BF16)
        tz[1:nrows[o]] = tabs[o][1:].astype(BF16)
        tz[nrows[o]:] = tab0[:, o * EMB:(o + 1) * EMB].astype(BF16)
        shared[f'tab{o+1}z'] = tz
    shared['iotaH'] = np.tile(np.arange(W1, dtype=np.float16)[None, :],
                              (P, 1))
    shared['eyeH'] = np.eye(P, dtype=np.float16)
    tab1p = np.zeros((W1, EMB), BF16)
    tab1p[:1005] = shared['tab1z']
    shared['tab1p'] = tab1p

    # ---- dedup words, sort by length (cnt1 desc), deal to cores
    x = np.asarray(inputs['x'], np.int64).reshape(-1)
    ux, inv = np.unique(x, return_inverse=True)
    key = meta[ux, CNTCOL]                     # cnt1 = word length
    order_u = np.argsort(-key, kind='stable')  # descending
    su = ux[order_u]                           # sorted unique words
    n_u = len(su)
    n_pad = -(-n_u // (NCORES * P)) * (NCORES * P)
    su = np.concatenate([su, np.zeros(n_pad - n_u, np.int64)])
    Gc = n_pad // (NCORES * P)                 # groups per core

    core_words = [su[c::NCORES] for c in range(NCORES)]

    profile = []
    for g in range(Gc):
        mx = [1, 1, 1]
        for c in range(NCORES):
            seg = core_words[c][g * P:(g + 1) * P]
            for o in range(3):
                mx[o] = max(mx[o], int(meta[seg, CNTCOL + o].max()))
        profile.append(tuple(mx))
    profile = tuple(profile)

    # token t -> rank of its word in su -> (core r%8, row r//8)
    rank_of = np.empty(n_u, np.int64)
    rank_of[order_u] = np.arange(n_u)
    tok_rank = rank_of[inv]

    in_maps = []
    for c in range(NCORES):
        m = dict(shared)
        m['xpg'] = np.ascontiguousarray(
            core_words[c].reshape(Gc, P).T).astype(np.int32)
        m['metaT'] = np.ascontiguousarray(
            meta[core_words[c]].reshape(Gc, P, MW)
            .transpose(1, 0, 2).reshape(P, Gc * MW))
        in_maps.append(m)
    return in_maps, profile, tok_rank

def _run(nc, in_maps, trace=False):
    return run_bass_kernel_spmd(nc, in_maps, core_ids=list(range(NCORES)),
                                trace=trace)


def kernel(**inputs):
    in_maps, profile, tok_rank = _prep(inputs)
    nc = _get_nc(profile)
    res = _run(nc, in_maps)
    by_rank = np.stack([np.asarray(res.results[c]['out'])
                        for c in range(NCORES)])      # [core, row, 768]
    out = by_rank[tok_rank % NCORES, tok_rank // NCORES]
    return out.reshape(B, S, 768)



# revision 4
# speedup vs baseline: 2.0698x; 2.0698x over previous
"""N-gram embedding lookup kernel for Trainium2 (8 NeuronCores, Bass/Tile).

Problem: for each token x[b,s] (vocab 50000), gather precomputed n-gram
hash ids for orders 1..3 (12+11+10 slots), gather embedding rows from
three tables (1001/10001/50001 x 256 fp32), masked-mean each order,
concat to 768 dims; tokens x<4 take tab0[x] instead.

Design v2 (bulk-gather ucode):
 - host: dedup x to unique words, sort by word length (cnt1 desc), deal
   round-robin to the 8 cores => every 128-word group holds words of
   nearly equal length; per-group slot counts (the "profile") shrink
   from (12,11,10) to the group's actual max cnt per order.
 - host: per core, collect the unique (order, hash-id) pairs its words
   reference and build a COMPACTED per-core table (row-sharded
   vocab-parallel tables, data-aware assignment): tabC[0]=zeros,
   tabC[1+j]=row of the j-th unique key. ~17k rows < int16 range, so
   one combined index space serves all three orders.
 - chip: per group of 128 words, InstDMAGatherAnt (gpsimd.dma_gather)
   bulk-gathers all slot rows [word-partition, slot-col, 256] bf16 in
   <=8-column (1024-index) chunks sprayed round-robin over the 4 SWDGE
   queues (measured ~4ns/row vs 12ns/row for walrus indirect DMA; >8
   cols per instruction overflows the descriptor carveout).
 - chip: DVE reduces each order's column range (bf16 -> f32) and
   multiplies by host-computed 1/cnt; SP stores [128,768] per group.
 - no PE/PSUM/histograms; Pool runs ~4ns/row descriptor gen, DVE and
   stores hide underneath.
"""
import numpy as np
import ml_dtypes
from contextlib import ExitStack

from concourse import bacc, bass, mybir, tile
from concourse.bass_utils import run_bass_kernel_spmd

BF16 = ml_dtypes.bfloat16

NCORES = 8
B, S = 8, 2048
P = 128
EMB = 256
V = 50000
LS = (12, 11, 10)
NQ = 4                        # SWDGE queues (ucode max)
CHUNK = 8                     # slot-columns per dma_gather (1024 idxs)
ROT = 5                       # slab rotation depth
TABC_ROWS = 20480             # per-core compacted table rows (padded)
NROWS = (1005, 10005, 50005)  # rows in tab{1,2,3}z incl. 4 special rows
OFF = (0, 1005, 11010)        # combined key-space offsets
TABALL = 61015


def _build(profile, unroll=1):
    """profile: tuple per group of (L1, L2, L3) column counts."""
    G = len(profile)
    TPC = G * P
    i16, f32, bf16 = mybir.dt.int16, mybir.dt.float32, mybir.dt.bfloat16
    nc = bacc.Bacc("TRN2", target_bir_lowering=False, debug=False,
                   num_devices=NCORES, num_swdge_queues=NQ)

    cols_g = [sum(p) for p in profile]
    colsmax = max(cols_g)
    W = sum(cols_g) * 8            # idx i16 elements per partition row

    d_tabc = nc.dram_tensor("tabc", [TABC_ROWS, EMB], bf16,
                            kind="ExternalInput")
    d_idx = nc.dram_tensor("idxs", [P, W], i16, kind="ExternalInput")
    d_rcp = nc.dram_tensor("rcp", [P, G * 3], f32, kind="ExternalInput")
    d_out = nc.dram_tensor("out", [TPC, 768], f32, kind="ExternalOutput")

    qctr = [0]

    with ExitStack() as ctx:
        tc = ctx.enter_context(tile.TileContext(nc))
        pool = ctx.enter_context(tc.tile_pool(name="sbuf", bufs=1))

        t_idx = pool.tile([P, W], i16)
        t_rcp = pool.tile([P, G * 3], f32)
        t_out = pool.tile([P, G * 768], f32)
        slabs = [pool.tile([P, colsmax * EMB], bf16, name=f"slab{i}")
                 for i in range(ROT)]

        if unroll > 1:
            # hardware loop for benchmarking: body is idempotent
            ctx.enter_context(tc.For_i(0, unroll))

        nc.sync.dma_start(out=t_idx[:], in_=d_idx[:])
        nc.sync.dma_start(out=t_rcp[:], in_=d_rcp[:])

        idx_off = [0]

        def emit_gather(g):
            slab = slabs[g % ROT]
            cols = cols_g[g]
            c0 = 0
            while c0 < cols:
                cc = min(CHUNK, cols - c0)
                n = cc * P
                nc.gpsimd.dma_gather(
                    out_ap=bass.AP(slab[:].tensor, c0 * EMB,
                                   [slab[:].ap[0], [EMB, cc], [1, EMB]]),
                    in_ap=d_tabc[:],
                    idxs_ap=t_idx[:, idx_off[0]:idx_off[0] + cc * 8],
                    num_idxs=n,
                    num_idxs_reg=n,
                    elem_size=EMB,
                    queue_num=qctr[0] % NQ,
                )
                qctr[0] += 1
                idx_off[0] += cc * 8
                c0 += cc

        def emit_tail(g):
            slab = slabs[g % ROT]
            L1, L2, L3 = profile[g]
            a = 0
            for o, L in enumerate((L1, L2, L3)):
                out_ap = t_out[:, g * 768 + o * 256:g * 768 + (o + 1) * 256]
                in1 = bass.AP(t_rcp[:].tensor, g * 3 + o,
                              [t_rcp[:].ap[0], [0, 256]])
                if L == 1:
                    nc.vector.tensor_tensor(
                        out=out_ap, in0=slab[:, a * EMB:(a + 1) * EMB],
                        in1=in1, op=mybir.AluOpType.mult)
                else:
                    nc.vector.tensor_reduce(
                        out=out_ap,
                        in_=bass.AP(slab[:].tensor, a * EMB,
                                    [slab[:].ap[0], [1, EMB], [EMB, L]]),
                        axis=mybir.AxisListType.X,
                        op=mybir.AluOpType.add, opt_input=False)
                    nc.vector.tensor_tensor(out=out_ap, in0=out_ap, in1=in1,
                                            op=mybir.AluOpType.mult)
                a += L
            nc.sync.dma_start(
                out=bass.AP(d_out, g * P * 768, [[768, P], [1, 768]]),
                in_=t_out[:, g * 768:(g + 1) * 768])

        for g in range(G):
            emit_gather(g)
            if g >= 2:
                emit_tail(g - 2)
        for g in range(max(G - 2, 0), G):
            emit_tail(g)

    return nc


_NC_CACHE = {}


def _get_nc(profile, nq=NQ, unroll=1, **kw):
    key = (profile, nq, unroll, tuple(sorted(kw.items())))
    if key not in _NC_CACHE:
        nc = _build(profile, unroll=unroll)
        nc.finalize()
        _NC_CACHE[key] = nc
    return _NC_CACHE[key]


def _prep(inputs):
    tab0 = np.asarray(inputs['tab0'], np.float32)
    tabs = [np.asarray(inputs[f'tab{o+1}'], np.float32) for o in range(3)]
    ids = [np.asarray(inputs[f'ids{o+1}'], np.int64) for o in range(3)]
    cnt = [np.asarray(inputs[f'cnt{o+1}'], np.int64) for o in range(3)]

    # ids/cnt per word with specials folded in: word v<4 -> slot0 points at
    # an appended per-special row, cnt 1
    idsw = []
    cntw = []
    for o in range(3):
        a = ids[o].astype(np.int64).copy()
        c = cnt[o].astype(np.int64).copy()
        a[:4] = 0
        a[:4, 0] = NROWS[o] - 4 + np.arange(4)
        c[:4] = 1
        idsw.append(a)
        cntw.append(c)

    # combined bf16 source table: [tab1z; tab2z; tab3z], each with row0=0,
    # rows 1..V real, last 4 rows the tab0 special slices
    taball = np.zeros((TABALL, EMB), BF16)
    for o in range(3):
        nz = NROWS[o]
        taball[OFF[o] + 1:OFF[o] + nz - 4] = tabs[o][1:].astype(BF16)
        taball[OFF[o] + nz - 4:OFF[o] + nz] = \
            tab0[:, o * EMB:(o + 1) * EMB].astype(BF16)

    # ---- dedup words, sort by length (cnt1 desc), deal to cores
    x = np.asarray(inputs['x'], np.int64).reshape(-1)
    ux, inv = np.unique(x, return_inverse=True)
    order_u = np.argsort(-cntw[0][ux], kind='stable')
    su = ux[order_u]
    n_u = len(su)
    n_pad = -(-n_u // (NCORES * P)) * (NCORES * P)
    su = np.concatenate([su, np.zeros(n_pad - n_u, np.int64)])
    Gc = n_pad // (NCORES * P)

    core_words = [su[c::NCORES].reshape(Gc, P) for c in range(NCORES)]

    # shared profile: per-group max cnt per order across cores
    profile = []
    for g in range(Gc):
        mx = []
        for o in range(3):
            m = 1
            for c in range(NCORES):
                m = max(m, int(cntw[o][core_words[c][g]].max()))
            mx.append(m)
        profile.append(tuple(mx))
    profile = tuple(profile)

    # token -> (core, row) mapping
    rank_of = np.empty(n_u, np.int64)
    rank_of[order_u] = np.arange(n_u)
    tok_rank = rank_of[inv]

    in_maps = []
    for c in range(NCORES):
        words = core_words[c]                          # [Gc, P]
        # combined keys per (group, slot-col): 0 = padding
        key_cols = []                                  # list of [P] arrays
        for g in range(Gc):
            wg = words[g]
            for o in range(3):
                L = profile[g][o]
                idg = idsw[o][wg][:, :L]               # [P, L]
                vmask = np.arange(L)[None, :] < cntw[o][wg][:, None]
                keys = np.where(vmask, idg + OFF[o], -1)   # -1 = padding
                key_cols.append(keys.T)                # [L, P]
        allk = np.concatenate([k.reshape(-1) for k in key_cols])
        uk = np.unique(allk[allk >= 0])
        assert len(uk) + 1 <= TABC_ROWS, f"{len(uk)=}"
        tabc = np.zeros((TABC_ROWS, EMB), BF16)
        tabc[1:1 + len(uk)] = taball[uk]
        # local index: 1 + rank in uk; padding -> 0
        idx_cols = []
        for kcol in key_cols:                          # [L, P] each
            loc = np.where(kcol >= 0,
                           1 + np.searchsorted(uk, np.maximum(kcol, 0)), 0)
            idx_cols.append(loc.astype(np.int16))
        flat = np.concatenate([k.reshape(-1) for k in idx_cols])  # col-major
        # dma_gather idx grid: index i of an instruction lives at
        # [16*blk + i%16, i//16]; chunks of <=8 columns are consecutive in
        # the flat stream, so the whole stream maps uniformly because every
        # chunk length is a multiple of 128.
        grid16 = flat.reshape(-1, 16).T                # [16, W]
        grid = np.tile(grid16, (8, 1))                 # [128, W]

        rcps = []
        for g in range(Gc):
            wg = words[g]
            r = np.stack([1.0 / cntw[o][wg] for o in range(3)], 1)  # [P,3]
            rcps.append(r)
        rcp_grid = np.concatenate(rcps, 1).astype(np.float32)  # [P, Gc*3]

        in_maps.append({
            'tabc': tabc,
            'idxs': np.ascontiguousarray(grid),
            'rcp': np.ascontiguousarray(rcp_grid),
        })
    return in_maps, profile, tok_rank


def _run(nc, in_maps, trace=False):
    return run_bass_kernel_spmd(nc, in_maps, core_ids=list(range(NCORES)),
                                trace=trace)


def kernel(**inputs):
    in_maps, profile, tok_rank = _prep(inputs)
    nc = _get_nc(profile)
    res = _run(nc, in_maps)
    by_rank = np.stack([np.asarray(res.results[c]['out'])
                        for c in range(NCORES)])      # [core, row, 768]
    out = by_rank[tok_rank % NCORES, tok_rank // NCORES]
    return out.reshape(B, S, 768)


# revision 34
# speedup vs baseline: 5.3256x; 2.5730x over previous
"""N-gram embedding lookup kernel for Trainium2 (8 NeuronCores, Bass/Tile).

Problem: for each token x[b,s] (vocab 50000), gather precomputed n-gram
hash ids for orders 1..3 (12+11+10 slots), gather embedding rows from
three tables (1001/10001/50001 x 256 fp32), masked-mean each order,
concat to 768 dims; tokens x<4 take tab0[x] instead.

Design v4 (count-matmul for orders 1/2, bulk-gather ucode for order 3):
 - KEY STRUCTURE: order-1 grams are single characters => <=26 distinct
   hash ids ever occur; order-2 grams are character pairs => <=676
   distinct ids. So sum_s tab[id_s] = counts @ T where counts is a tiny
   per-word histogram (pure index data, built on host) and T is the
   table restricted to the occurring ids. Orders 1+2 become one K=32
   and one K=768 PE matmul per 128-word group - no gather descriptors
   at all. Order-3 (26^3 = 17576 possible ids) stays a real gather.
 - host: dedup x to unique words, sort by word length, deal round-robin
   to 8 cores => each 128-word group has near-uniform slot counts.
 - host: per core, compact the order-3 rows actually referenced into a
   per-core table (row-sharded vocab-parallel, data-aware): tabC[0]=0,
   tabC[1+j] = j-th unique row; ~9k rows, int16-indexable.
 - chip order 3: per group, gpsimd.dma_gather (InstDMAGatherAnt) bulk-
   gathers slot rows [word-partition, slot-col, 256] bf16 in <=8-column
   (1024-index) chunks round-robin over the 4 SWDGE queues (~4ns/row;
   >1024 indexes per instruction overflows the ucode descriptor ring).
 - chip orders 1+2: PE matmuls of host-built count matrices against
   SBUF-resident letter/bigram tables, accumulated in PSUM.
 - DVE reduces the order-3 slot columns (bf16 -> f32); ACT applies the
   1/cnt scales and drains PSUM; SP stores [128,768] per group.
"""
import numpy as np
import ml_dtypes
from contextlib import ExitStack

from concourse import bacc, bass, mybir, tile
from concourse.bass_utils import run_bass_kernel_spmd

BF16 = ml_dtypes.bfloat16

NCORES = 8
B, S = 8, 2048
P = 128
EMB = 256
V = 50000
NQ = 4                        # SWDGE queues (ucode max)
CHUNK = 8                     # slot-columns per dma_gather (1024 descs max)
ROT = 5                       # slab rotation depth
PSROT = 4                     # psum rotation depth per order
TABC_ROWS = 12288             # per-core compacted order-3 table rows
NROWS = (1005, 10005, 50005)  # rows in tab{1,2,3}z incl. 4 special rows
K1 = 32                       # order-1 id space (<=26 letters + 4 specials)
K2 = 768                      # order-2 id space (<=676 bigrams + 4 specials)
NB2 = K2 // P


def _build(profile, unroll=1, chunk=CHUNK, rot=ROT, parts="gmts"):
    """profile: tuple per group of (L3,) max order-3 slot counts.
    parts: g=gathers m=matmuls t=tails s=stores (debug decomposition)."""
    G = len(profile)
    TPC = G * P
    i16, f32, bf16 = mybir.dt.int16, mybir.dt.float32, mybir.dt.bfloat16
    nc = bacc.Bacc("TRN2", target_bir_lowering=False, debug=False,
                   num_devices=NCORES, num_swdge_queues=NQ)

    cols_g = [p[0] for p in profile]
    TOT = sum(cols_g)              # total slot columns in the flat slab
    acol = np.cumsum([0] + cols_g)  # per-group column offsets
    W = TOT * 8                    # idx i16 elements per partition row

    d_tabc = nc.dram_tensor("tabc", [TABC_ROWS, EMB], bf16,
                            kind="ExternalInput")
    d_idx = nc.dram_tensor("idxs", [P, W], i16, kind="ExternalInput")
    d_rcp = nc.dram_tensor("rcp", [P, G * 3], f32, kind="ExternalInput")
    d_t1 = nc.dram_tensor("t1r", [K1, EMB], bf16, kind="ExternalInput")
    d_t2 = nc.dram_tensor("t2r", [P, NB2 * EMB], bf16, kind="ExternalInput")
    d_lc1 = nc.dram_tensor("lc1", [K1, G * P], bf16, kind="ExternalInput")
    d_lc2 = nc.dram_tensor("lc2", [P, NB2 * G * P], bf16,
                           kind="ExternalInput")
    d_out = nc.dram_tensor("out", [TPC, 768], f32, kind="ExternalOutput")

    qctr = [0]

    with ExitStack() as ctx:
        tc = ctx.enter_context(tile.TileContext(nc))
        pool = ctx.enter_context(tc.tile_pool(name="sbuf", bufs=1))
        psum = ctx.enter_context(
            tc.tile_pool(name="psum", bufs=1, space="PSUM"))

        t_idx = pool.tile([P, W], i16)
        t_rcp = pool.tile([P, G * 3], f32)
        t_out = pool.tile([P, G * 768], f32)
        t_t1 = pool.tile([P, EMB], bf16)
        t_t2 = pool.tile([P, NB2 * EMB], bf16)
        t_lc1 = pool.tile([P, G * P], bf16)
        t_lc2 = pool.tile([P, NB2 * G * P], bf16)
        colsmax = max(cols_g)
        slabs = [pool.tile([P, colsmax * EMB], bf16, name=f"slab{i}")
                 for i in range(rot)]
        ps1 = [psum.tile([P, EMB], f32, name=f"ps1_{i}")
               for i in range(PSROT)]
        ps2 = [psum.tile([P, EMB], f32, name=f"ps2_{i}")
               for i in range(PSROT)]

        # small tables: load once (weights-like)
        nc.sync.dma_start(out=t_t1[0:K1, :], in_=d_t1[:])
        nc.sync.dma_start(out=t_t2[:], in_=d_t2[:])

        if unroll > 1:
            # hardware loop for benchmarking: body is idempotent
            ctx.enter_context(tc.For_i(0, unroll))

        # split the idx load so group 0's gathers unblock immediately
        w0 = cols_g[0] * 8
        nc.sync.dma_start(out=t_idx[:, 0:w0], in_=d_idx[:, 0:w0])
        nc.sync.dma_start(out=t_idx[:, w0:], in_=d_idx[:, w0:])
        nc.sync.dma_start(out=t_rcp[:], in_=d_rcp[:])
        nc.scalar.dma_start(out=t_lc1[0:K1, :], in_=d_lc1[:])
        # per-chunk lc2 loads so group 0's matmuls unblock chunk by chunk
        for k in range(NB2):
            nc.scalar.dma_start(
                out=t_lc2[:, k * G * P:(k + 1) * G * P],
                in_=d_lc2[:, k * G * P:(k + 1) * G * P])

        idx_off = [0]

        def emit_gather(g):
            slab = slabs[g % rot]
            cols = cols_g[g]
            c0 = 0
            while c0 < cols:
                cc = min(chunk, cols - c0)
                n = cc * P
                nc.gpsimd.dma_gather(
                    out_ap=bass.AP(slab[:].tensor, c0 * EMB,
                                   [slab[:].ap[0], [EMB, cc], [1, EMB]]),
                    in_ap=d_tabc[:],
                    idxs_ap=t_idx[:, idx_off[0]:idx_off[0] + cc * 8],
                    num_idxs=n,
                    num_idxs_reg=n,
                    elem_size=EMB,
                    queue_num=qctr[0] % NQ,
                )
                qctr[0] += 1
                idx_off[0] += cc * 8
                c0 += cc

        def emit_matmuls(g):
            nc.tensor.matmul(
                ps1[g % PSROT][:],
                lhsT=t_lc1[0:K1, g * P:(g + 1) * P],
                rhs=t_t1[0:K1, :],
                start=True, stop=True)
            pB = ps2[g % PSROT]
            for k in range(NB2):
                nc.tensor.matmul(
                    pB[:],
                    lhsT=t_lc2[:, (k * G + g) * P:(k * G + g + 1) * P],
                    rhs=t_t2[:, k * EMB:(k + 1) * EMB],
                    start=(k == 0), stop=(k == NB2 - 1))

        def emit_tail(g):
            slab = slabs[g % rot]
            L3 = profile[g][0]
            nc.scalar.mul(t_out[:, g * 768:g * 768 + 256],
                          ps1[g % PSROT][:], t_rcp[:, g * 3:g * 3 + 1])
            nc.scalar.mul(t_out[:, g * 768 + 256:g * 768 + 512],
                          ps2[g % PSROT][:], t_rcp[:, g * 3 + 1:g * 3 + 2])
            out_ap = t_out[:, g * 768 + 512:g * 768 + 768]
            rcp_ap = t_rcp[:, g * 3 + 2:g * 3 + 3]
            if L3 == 1:
                nc.scalar.mul(out_ap, slab[:, 0:EMB], rcp_ap)
            else:
                nc.vector.tensor_reduce(
                    out=out_ap,
                    in_=bass.AP(slab[:].tensor, 0,
                                [slab[:].ap[0], [1, EMB], [EMB, L3]]),
                    axis=mybir.AxisListType.X,
                    op=mybir.AluOpType.add, opt_input=False)
                nc.scalar.mul(out_ap, out_ap, rcp_ap)
            if "s" in parts:
                nc.sync.dma_start(
                    out=bass.AP(d_out, g * P * 768, [[768, P], [1, 768]]),
                    in_=t_out[:, g * 768:(g + 1) * 768])

        for g in range(G):
            if "g" in parts:
                emit_gather(g)
            if "m" in parts:
                emit_matmuls(g)
            if g >= 2 and "t" in parts:
                emit_tail(g - 2)
        if "t" in parts:
            for g in range(max(G - 2, 0), G):
                emit_tail(g)

    return nc


_NC_CACHE = {}


def _get_nc(profile, nq=NQ, unroll=1, **kw):
    key = (profile, nq, unroll, tuple(sorted(kw.items())))
    if key not in _NC_CACHE:
        nc = _build(profile, unroll=unroll, **kw)
        nc.finalize()
        _NC_CACHE[key] = nc
    return _NC_CACHE[key]


def _prep(inputs):
    tab0 = np.asarray(inputs['tab0'], np.float32)
    tabs = [np.asarray(inputs[f'tab{o+1}'], np.float32) for o in range(3)]
    ids = [np.asarray(inputs[f'ids{o+1}'], np.int64) for o in range(3)]
    cnt = [np.asarray(inputs[f'cnt{o+1}'], np.int64) for o in range(3)]

    # ids/cnt per word with specials folded in: word v<4 -> slot0 points at
    # an appended per-special row, cnt 1
    idsw = []
    cntw = []
    for o in range(3):
        a = ids[o].astype(np.int64).copy()
        c = cnt[o].astype(np.int64).copy()
        a[:4] = 0
        a[:4, 0] = NROWS[o] - 4 + np.arange(4)
        c[:4] = 1
        idsw.append(a)
        cntw.append(c)

    # per-order tables with row0=0, rows 1..V real, +4 special rows
    tabz = []
    for o in range(3):
        nz = NROWS[o]
        tz = np.zeros((nz, EMB), BF16)
        tz[1:nz - 4] = tabs[o][1:].astype(BF16)
        tz[nz - 4:] = tab0[:, o * EMB:(o + 1) * EMB].astype(BF16)
        tabz.append(tz)

    # ---- orders 1/2: global id spaces (<=26+4 and <=676+4 distinct)
    uid = []
    for o in range(2):
        vals = idsw[o].reshape(-1)
        msk = (np.arange(idsw[o].shape[1])[None, :]
               < cntw[o][:, None]).reshape(-1)
        u = np.unique(vals[msk])
        uid.append(u)
    assert len(uid[0]) <= K1 and len(uid[1]) <= K2, \
        (len(uid[0]), len(uid[1]))
    t1r = np.zeros((K1, EMB), BF16)
    t1r[:len(uid[0])] = tabz[0][uid[0]]
    t2r = np.zeros((K2, EMB), BF16)
    t2r[:len(uid[1])] = tabz[1][uid[1]]
    # PE rhs layout: [128, NB2*EMB] with chunk k at cols k*EMB..
    t2r_pack = np.zeros((P, NB2 * EMB), BF16)
    for k in range(NB2):
        t2r_pack[:, k * EMB:(k + 1) * EMB] = t2r[k * P:(k + 1) * P]

    shared = {'t1r': t1r, 't2r': t2r_pack}

    # ---- dedup words, sort by length (cnt1 desc), deal to cores
    x = np.asarray(inputs['x'], np.int64).reshape(-1)
    ux, inv = np.unique(x, return_inverse=True)
    order_u = np.argsort(-cntw[0][ux], kind='stable')
    su = ux[order_u]
    n_u = len(su)
    n_pad = -(-n_u // (NCORES * P)) * (NCORES * P)
    su = np.concatenate([su, np.zeros(n_pad - n_u, np.int64)])
    Gc = n_pad // (NCORES * P)

    core_words = [su[c::NCORES].reshape(Gc, P) for c in range(NCORES)]

    # shared profile: per-group max order-3 cnt across cores
    profile = []
    for g in range(Gc):
        m = 1
        for c in range(NCORES):
            m = max(m, int(cntw[2][core_words[c][g]].max()))
        profile.append((m,))
    profile = tuple(profile)

    # per-word count rows over the order-1/2 id spaces (pure index data)
    def count_matrix(o, K, words_flat):
        nw = len(words_flat)
        lc = np.zeros((nw, K), np.int16)
        idg = idsw[o][words_flat]                      # [nw, L]
        L = idg.shape[1]
        vm = np.arange(L)[None, :] < cntw[o][words_flat][:, None]
        rows = np.repeat(np.arange(nw), L).reshape(nw, L)[vm]
        cols = np.searchsorted(uid[o], idg[vm])
        np.add.at(lc, (rows, cols), 1)
        return lc

    # token -> (core, row) mapping
    rank_of = np.empty(n_u, np.int64)
    rank_of[order_u] = np.arange(n_u)
    tok_rank = rank_of[inv]

    in_maps = []
    for c in range(NCORES):
        words = core_words[c]                          # [Gc, P]
        wflat = words.reshape(-1)

        lc1 = count_matrix(0, K1, wflat)               # [Gc*P, K1]
        lc2 = count_matrix(1, K2, wflat)               # [Gc*P, K2]
        lc1t = np.ascontiguousarray(lc1.T).astype(BF16)    # [K1, Gc*P]
        # lc2 lhsT chunks: [128, NB2*Gc*P], chunk k group g at
        # cols (k*Gc+g)*P ..
        lc2t = np.zeros((P, NB2 * Gc * P), BF16)
        for k in range(NB2):
            blk = lc2[:, k * P:(k + 1) * P].T          # [P, Gc*P]
            lc2t[:, k * Gc * P:(k + 1) * Gc * P] = blk.astype(BF16)

        # ---- order-3 compacted per-core table + idx stream
        key_cols = []
        for g in range(Gc):
            wg = words[g]
            L = profile[g][0]
            idg = idsw[2][wg][:, :L]
            vmask = np.arange(L)[None, :] < cntw[2][wg][:, None]
            keys = np.where(vmask, idg, -1)
            key_cols.append(keys.T)                    # [L, P]
        allk = np.concatenate([k.reshape(-1) for k in key_cols])
        uk = np.unique(allk[allk >= 0])
        assert len(uk) + 1 <= TABC_ROWS, f"{len(uk)=}"
        tabc = np.zeros((TABC_ROWS, EMB), BF16)
        tabc[1:1 + len(uk)] = tabz[2][uk]
        idx_cols = []
        for kcol in key_cols:
            loc = np.where(kcol >= 0,
                           1 + np.searchsorted(uk, np.maximum(kcol, 0)), 0)
            idx_cols.append(loc.astype(np.int16))
        flat = np.concatenate([k.reshape(-1) for k in idx_cols])
        grid16 = flat.reshape(-1, 16).T
        grid = np.tile(grid16, (8, 1))                 # [128, W]

        rcps = []
        for g in range(Gc):
            wg = words[g]
            r = np.stack([1.0 / cntw[o][wg] for o in range(3)], 1)
            rcps.append(r)
        rcp_grid = np.concatenate(rcps, 1).astype(np.float32)

        m = dict(shared)
        m['tabc'] = tabc
        m['idxs'] = np.ascontiguousarray(grid)
        m['rcp'] = np.ascontiguousarray(rcp_grid)
        m['lc1'] = lc1t
        m['lc2'] = np.ascontiguousarray(lc2t)
        in_maps.append(m)
    return in_maps, profile, tok_rank


def _run(nc, in_maps, trace=False):
    return run_bass_kernel_spmd(nc, in_maps, core_ids=list(range(NCORES)),
                                trace=trace)


def kernel(**inputs):
    in_maps, profile, tok_rank = _prep(inputs)
    nc = _get_nc(profile)
    res = _run(nc, in_maps)
    by_rank = np.stack([np.asarray(res.results[c]['out'])
                        for c in range(NCORES)])      # [core, row, 768]
    out = by_rank[tok_rank % NCORES, tok_rank // NCORES]
    return out.reshape(B, S, 768)
